# revision 8
# baseline (speedup 1.0000x reference)
"""MultiLinearUpsampling Trainium2 kernel.

Problem: out[b, t, :] = W[lidx[t]] @ pooled[b, segc[t], :]  (zero for invalid t)
where segc/lidx derive from sorted pooling_indices (ragged segments).

Strategy
--------
Host computes the segment structure (searchsorted / lengths).  Only
sum_l N_l matvecs are unique per batch (N_l = #segments with len > l;
positions past offset L-1 in a segment all reuse the l=L-1 result), far
fewer than the dense N*L of the reference einsum.

Segments are sorted by length (desc), so the segments needing linear l
are exactly the first N_l columns of the sorted layout -> contiguous
matmuls.  Sharding: core c owns linears {c, c+8} (weights 8MB/core);
every core sees the full transposed+sorted activation tensor
XT (D, B, N) and computes, for phase p in {0,1}:

    y_p[dout, b, 0:C_p] = W[l].T.T @ XT[:, b, 0:C_p]    (l = c + 8p)

with C_p = max over cores of N_l for that phase (= N_{8p} since N_l is
non-increasing); cores with smaller N_l compute a few extra harmless
columns that the host discards.  Matmuls run in fp32r (full PE rate at
free-dim >= 256); batches are grouped so the moving free dim is
>= 256 where possible.  Host scatters the computed vectors to their t
positions (incl. the l=15 tail replication) and zero-fills invalid t.
"""

import math
from contextlib import ExitStack

import numpy as np

import concourse.bass as bass
import concourse.mybir as mybir
import concourse.tile as tile
from concourse import bacc
from concourse.bass_utils import run_bass_kernel_spmd

F32 = mybir.dt.float32
F32R = mybir.dt.float32r

B = 8          # batch (== n_cores; each l-pair core sees all batches)
N = 512        # segments
D = 1024       # D_in == D_out
L = 16         # linears
NCORES = 8
KC = 8         # contraction chunks of 128
MC = 8         # output-dim chunks of 128


def _phase_plan(C):
    """Batch-group size g so the matmul moving free dim g*C lands in
    [256, 512] when possible (fp32r full-rate needs >= 256)."""
    if C == 0:
        return 1
    return max(1, min(B, 512 // C))


def _build_program(C_phases):
    """One SPMD program, shapes baked from the (data-dependent) column
    counts per phase.  Inputs: xt (D, B, N) f32, wt (P, D, D) f32
    (wt[p] = W[l_p].T).  Outputs: y{p} (D, B, C_p) f32."""
    nc = bacc.Bacc("TRN2", target_bir_lowering=False, debug=False)
    P = len(C_phases)

    xt = nc.dram_tensor("xt", (D, B, N), F32R, kind="ExternalInput")
    wt = nc.dram_tensor("wt", (P, D, D), F32R, kind="ExternalInput")
    ys = [
        nc.dram_tensor(f"y{p}", (D, B, C), F32, kind="ExternalOutput")
        for p, C in enumerate(C_phases)
    ]

    xt_r = xt.ap().rearrange("(kc kp) b n -> kp kc b n", kp=128)
    wt_r = wt.ap().rearrange("p (kc kp) m -> kp p kc m", kp=128)

    with tile.TileContext(nc) as tc, ExitStack() as ctx:
        wpool = ctx.enter_context(tc.tile_pool(name="w", bufs=2))
        xpool = ctx.enter_context(tc.tile_pool(name="x", bufs=3))
        opool = ctx.enter_context(tc.tile_pool(name="o", bufs=4))
        ppool = ctx.enter_context(tc.tile_pool(name="ps", bufs=4, space="PSUM"))

        for p, C in enumerate(C_phases):
            if C == 0:
                continue
            wtile = wpool.tile([128, KC, D], F32R, tag="w")
            nc.sync.dma_start(wtile[:], wt_r[:, p])
            g = _phase_plan(C)
            for b0 in range(0, B, g):
                gg = min(g, B - b0)
                xtile = xpool.tile([128, KC, gg, C], F32R, tag="x")
                for k in range(KC):
                    nc.sync.dma_start(
                        xtile[:, k], xt_r[:, k, b0 : b0 + gg, 0:C]
                    )
                for m in range(MC):
                    ps = ppool.tile([128, gg, C], F32, tag="ps")
                    for k in range(KC):
                        nc.tensor.matmul(
                            ps[:],
                            wtile[:, k, m * 128 : (m + 1) * 128],
                            xtile[:, k],
                            start=(k == 0),
                            stop=(k == KC - 1),
                        )
                    ot = opool.tile([128, gg, C], F32, tag="o")
                    nc.vector.tensor_copy(ot[:], ps[:])
                    nc.sync.dma_start(
                        ys[p][m * 128 : (m + 1) * 128, b0 : b0 + gg, :], ot[:]
                    )

    nc.compile()
    return nc


def _segment_structure(idx, T):
    """Host replica of the reference's searchsorted mapping."""
    t = np.arange(T)
    seg = np.searchsorted(idx, t, side="left")
    valid = seg < N
    segc = np.clip(seg, 0, N - 1)
    start = np.where(segc > 0, idx[np.maximum(segc - 1, 0)] + 1, 0)
    lidx = np.minimum(t - start, L - 1).astype(np.int64)
    lens = np.bincount(segc[valid], minlength=N)
    return t, seg, valid, segc, lidx, lens


def _install_ntff_hook():
    """Profiling-only: register the axon NTFF profile hook that the boot
    path skips when antenv.axon_hooks is absent (dev harness use)."""
    import sys
    import types

    try:
        import antenv

        if "antenv.axon_hooks" not in sys.modules:
            mod = types.ModuleType("antenv.axon_hooks")
            holder = [None]
            mod.set_axon_ntff_profile_hook = lambda h: holder.__setitem__(0, h)
            mod.get_axon_ntff_profile_hook = lambda: holder[0]
            sys.modules["antenv.axon_hooks"] = mod
            antenv.axon_hooks = mod
            from trn_agent_boot.trn_boot import _ntff_profile_via_ctypes

            mod.set_axon_ntff_profile_hook(
                _ntff_profile_via_ctypes("/opt/axon/libaxon_pjrt.so")
            )
    except Exception as e:  # profiling is best-effort
        print(f"NTFF hook install failed: {e}")


def kernel(pooled_vectors, W, pooling_indices, target_length, _trace=False):
    pooled = np.asarray(pooled_vectors, dtype=np.float32)
    Wf = np.asarray(W, dtype=np.float32)
    idx = np.asarray(pooling_indices).astype(np.int64)
    T = int(np.asarray(target_length))

    t, seg, valid, segc, lidx, lens = _segment_structure(idx, T)

    # sort segments by covered length, descending (stable)
    order = np.argsort(-lens, kind="stable")
    rank_of_seg = np.empty(N, dtype=np.int64)
    rank_of_seg[order] = np.arange(N)
    N_l = (lens[None, :] > np.arange(L)[:, None]).sum(axis=1)  # (L,)

    # phase p of core c handles linear l = c + 8p; N_l non-increasing so
    # the program-wide column count per phase is N_{8p}.  fp32r matmuls
    # require an even moving free count -> round up (extra cols are
    # real-but-unneeded data the host discards).
    def _even(c):
        return min(N, c + (c % 2))

    C_phases = (_even(int(N_l[0])), _even(int(N_l[8])))

    nc = _build_program(C_phases)

    # inputs: replicated sorted-transposed activations + per-core weights
    xt_global = np.ascontiguousarray(pooled.transpose(2, 0, 1)[:, :, order])
    in_maps = []
    for c in range(NCORES):
        wt_c = np.ascontiguousarray(
            np.stack([Wf[c].T, Wf[c + 8].T]).astype(np.float32)
        )
        in_maps.append({"xt": xt_global, "wt": wt_c})

    kwargs = {}
    if _trace:
        _install_ntff_hook()
        kwargs = dict(trace=True)
    res = run_bass_kernel_spmd(nc, in_maps, core_ids=list(range(NCORES)), **kwargs)
    results = res.results

    # assemble: Yp[c] has columns for l = c + 8p
    Y0 = np.stack([results[c]["y0"] for c in range(NCORES)])  # (8, D, B, C1)
    Y1 = np.stack([results[c]["y1"] for c in range(NCORES)])  # (8, D, B, C2)

    Dout = Wf.shape[1]
    out = np.zeros((B, T, Dout), dtype=np.float32)
    tv = t[valid]
    l_t = lidx[valid]
    r_t = rank_of_seg[segc[valid]]
    m0 = l_t < 8
    if m0.any():
        out[:, tv[m0], :] = Y0[l_t[m0], :, :, r_t[m0]].transpose(2, 0, 1)
    if (~m0).any():
        out[:, tv[~m0], :] = Y1[l_t[~m0] - 8, :, :, r_t[~m0]].transpose(2, 0, 1)

    if _trace:
        kernel._last_exec_time_ns = res.exec_time_ns
        kernel._last_results = res
    return out
